# revision 6
# baseline (speedup 1.0000x reference)
"""Masked mean neighbor aggregation (GNN message passing) on 8 TRN2 cores.

Strategy (per spec sharding hint): batch dim sharded across 8 cores, feature
table replicated in each core's DRAM.

Device algorithm, per core, per 128-row output tile:
  - The table is stored in bf16 (host-converted; halves gather traffic, and
    one-hot matmuls run at full bf16 PE rate).  It is split into 4 row-range
    chunks (<=32768 rows each) so row indices fit dma_gather's int16 index
    format.  The host compacts each tile's unmasked (row, neighbor) slots by
    chunk into fixed-capacity flat index lists.
  - Pad slots beyond each group's actual count are -1: dma_gather skips
    trailing negative indices entirely (no descriptors, no HBM traffic).  The
    true per-group count is shipped in a small int32 tensor and loaded into a
    register (num_idxs_reg) right before each gather.  The first DEPTH tiles
    instead use full-capacity row-0 padding so every G buffer is fully
    initialized before its first reuse (stale data is then always finite;
    pad lanes are killed by the selection matrix, but NaN*0 would not be).
  - 4 dma_gather instructions (one per chunk, on 4 parallel SWDGE queues)
    pull the 256 B bf16 feature rows into an SBUF tile G.  4-queue
    round-robin keeps descriptor generation off the critical path.
  - The slots land in a data-dependent order, so the host also ships a tiny
    per-slot target-lane array; the vector engine expands it on device into
    one-hot bf16 selection matrices (pad slots get lane 128 -> all-zero
    column).
  - PE computes psum[b, d] = sum_slots sel[slot, b] * G[slot, d] with PSUM
    accumulation over the 16 slot-blocks: simultaneously the masked sum and
    the reordering (bf16 inputs, fp32 accumulate).
  - The scalar engine scales by 1/max(count,1) (host-precomputed) and the
    result is stored as fp32.

Everything is raw bacc with manual semaphores (the Tile layer does not know
dma_gather's DMA semantics).
"""

from contextlib import ExitStack

import numpy as np

N_NODES = 100000
D_FEAT = 128
BATCH = 50000
K = 25
N_CORES = 8
P = 128

N_CHUNKS = 4
CHUNK_ROWS = 25000           # N_NODES / N_CHUNKS, < 32768 so int16-safe
CAP = 512                    # per (tile, chunk) gather capacity, mult of 128
TILES_PER_CORE = 49          # ceil(50000 / 8 / 128)
B_LOC = TILES_PER_CORE * P   # 6272
B_PAD = B_LOC * N_CORES      # 50176
DEPTH = 8                    # software pipeline depth (G/SEL/PSUM buffers)
SKIP_PADS = False             # trailing -1 idx pads skip the DMA fetch

_prog_cache = {}


def _build_program(n_rows, chunk_rows, n_chunks, d, n_tiles, cap, reps=1):
    import concourse.bass as bass
    import concourse.bacc as bacc
    import concourse.mybir as mybir
    from concourse.library_config import mlp

    bpc = cap // P               # G column-blocks per chunk
    nblk = n_chunks * bpc        # selection blocks per tile
    ic = cap // 16               # idx columns per gather (wrapped int16)

    nc = bacc.Bacc("TRN2", target_bir_lowering=False, debug=False,
                   num_devices=N_CORES, num_swdge_queues=n_chunks)

    ftab = nc.dram_tensor("features", [n_rows, d], mybir.dt.bfloat16,
                          kind="ExternalInput")
    idx_d = nc.dram_tensor("idx", [P, n_tiles * n_chunks * ic],
                           mybir.dt.int16, kind="ExternalInput")
    b_d = nc.dram_tensor("bidx", [P, n_tiles * nblk], mybir.dt.bfloat16,
                         kind="ExternalInput")
    iota_d = nc.dram_tensor("iota", [P, P], mybir.dt.bfloat16,
                            kind="ExternalInput")
    winv_d = nc.dram_tensor("winv", [P, n_tiles], mybir.dt.float32,
                            kind="ExternalInput")
    cnt_d = nc.dram_tensor("cnt", [P, n_tiles * n_chunks], mybir.dt.int32,
                           kind="ExternalInput")
    out_d = nc.dram_tensor("out", [n_tiles * P, d], mybir.dt.float32,
                           kind="ExternalOutput")

    with ExitStack() as stack:
        block = stack.enter_context(nc.Block())
        ec = stack.enter_context
        idx_sb = ec(nc.sbuf_tensor("idx_sb", [P, n_tiles * n_chunks * ic],
                                   mybir.dt.int16))
        b_sb = ec(nc.sbuf_tensor("b_sb", [P, n_tiles * nblk],
                                 mybir.dt.bfloat16))
        iota_sb = ec(nc.sbuf_tensor("iota_sb", [P, P], mybir.dt.bfloat16))
        winv_sb = ec(nc.sbuf_tensor("winv_sb", [P, n_tiles],
                                    mybir.dt.float32))
        cnt_sb = ec(nc.sbuf_tensor("cnt_sb", [P, n_tiles * n_chunks],
                                   mybir.dt.int32))
        G = [ec(nc.sbuf_tensor(f"g{r}", [P, nblk, d], mybir.dt.bfloat16))
             for r in range(DEPTH)]
        SEL = [ec(nc.sbuf_tensor(f"sel{r}", [P, nblk, P], mybir.dt.bfloat16))
               for r in range(DEPTH)]
        OSB = [ec(nc.sbuf_tensor(f"osb{r}", [P, d], mybir.dt.float32))
               for r in range(DEPTH)]
        PS = [ec(nc.psum_tensor(f"ps{r}", [P, d], mybir.dt.float32))
              for r in range(DEPTH)]
        io = ec(nc.semaphore("io"))
        gq = [[ec(nc.semaphore(f"gq{c}_{r}")) for r in range(DEPTH)]
              for c in range(n_chunks)]
        selg = ec(nc.semaphore("selg"))
        mmd = ec(nc.semaphore("mmd"))
        scd = ec(nc.semaphore("scd"))
        sto = [ec(nc.semaphore(f"sto{r}")) for r in range(DEPTH)]

        @block.sync
        def _(sync: bass.BassEngine):
            sync.dma_start(idx_sb[:], idx_d[:]).then_inc(io, 16)
            sync.dma_start(b_sb[:], b_d[:]).then_inc(io, 16)
            sync.dma_start(iota_sb[:], iota_d[:]).then_inc(io, 16)
            sync.dma_start(winv_sb[:], winv_d[:]).then_inc(io, 16)
            sync.dma_start(cnt_sb[:], cnt_d[:]).then_inc(io, 16)
            for tau in range(reps * n_tiles):
                t = tau % n_tiles
                sync.wait_ge(scd, tau + 1)
                sync.dma_start(out_d[t * P:(t + 1) * P, :],
                               OSB[tau % DEPTH][:]).then_inc(
                                   sto[tau % DEPTH], 16)

        @block.gpsimd
        def _(gpsimd: bass.BassGpSimd):
            gpsimd.load_library(mlp)
            gpsimd.wait_ge(io, 80)
            for tau in range(reps * n_tiles):
                t = tau % n_tiles
                if tau >= DEPTH:
                    gpsimd.wait_ge(mmd, tau - DEPTH + 1)  # G[tau%DEPTH] free
                gt = G[tau % DEPTH]
                for c in range(n_chunks):
                    g = t * n_chunks + c
                    if SKIP_PADS:
                        nreg = gpsimd.value_load(cnt_sb[0:1, g:g + 1],
                                                 min_val=1, max_val=cap)
                    else:
                        nreg = cap
                    src = ftab[c * chunk_rows:(c + 1) * chunk_rows, :]
                    idxs = idx_sb[:, g * ic:(g + 1) * ic]
                    gpsimd.dma_gather(
                        gt[:, c * bpc:(c + 1) * bpc, :], src, idxs,
                        cap, nreg, d, queue_num=c,
                    ).then_inc(gq[c][tau % DEPTH], 16)

        @block.vector
        def _(vector: bass.BassVectorEngine):
            vector.wait_ge(io, 80)
            iv = iota_sb.ap()
            iota_bc = bass.AP(iv.tensor, iv.offset,
                              [iv.ap[0], [0, nblk], iv.ap[1]])
            for tau in range(reps * n_tiles):
                t = tau % n_tiles
                if tau >= DEPTH:
                    vector.wait_ge(mmd, tau - DEPTH + 1)  # sel free
                st = SEL[tau % DEPTH]
                bv = b_sb[:, t * nblk:(t + 1) * nblk]
                b_bc = bass.AP(bv.tensor, bv.offset,
                               [bv.ap[0], bv.ap[1], [0, P]])
                vector.tensor_tensor(
                    out=st[:], in0=iota_bc, in1=b_bc,
                    op=mybir.AluOpType.is_equal,
                ).then_inc(selg, 1)

        @block.scalar
        def _(scalar: bass.BassEngine):
            scalar.wait_ge(io, 80)
            for tau in range(reps * n_tiles):
                t = tau % n_tiles
                scalar.wait_ge(mmd, tau + 1)     # psum[tau%DEPTH] ready
                if tau >= DEPTH:
                    scalar.wait_ge(sto[tau % DEPTH],
                                   16 * (tau // DEPTH))  # OSB free
                scalar.mul(OSB[tau % DEPTH][:], PS[tau % DEPTH][:],
                           winv_sb[:, t:t + 1]).then_inc(scd, 1)

        @block.tensor
        def _(tensor: bass.BassEngine):
            for tau in range(reps * n_tiles):
                for c in range(n_chunks):
                    tensor.wait_ge(gq[c][tau % DEPTH],
                                   16 * (tau // DEPTH + 1))
                tensor.wait_ge(selg, tau + 1)
                if tau >= DEPTH:
                    tensor.wait_ge(scd, tau - DEPTH + 1)  # psum drained
                gt, st, pst = (G[tau % DEPTH], SEL[tau % DEPTH],
                               PS[tau % DEPTH])
                for blk in range(nblk):
                    inst = nc.tensor.matmul(
                        pst[:], st[:, blk, :], gt[:, blk, :],
                        start=(blk == 0), stop=(blk == nblk - 1),
                    )
                inst.then_inc(mmd, 1)

    nc.compile()
    return nc


def get_program(reps=1):
    key = (N_NODES, CHUNK_ROWS, N_CHUNKS, D_FEAT, TILES_PER_CORE, CAP, reps)
    if key not in _prog_cache:
        _prog_cache[key] = _build_program(*key)
    return _prog_cache[key]


def pack_core(midx, mask, winv, n_tiles, cap, chunk_rows, n_chunks):
    """Compact one core's slots into gather/selection/count arrays.

    midx: [b_loc, K] int32 global row idx; mask: [b_loc, K] bool;
    winv: [b_loc] f32.
    """
    import ml_dtypes

    bpc = cap // P
    nblk = n_chunks * bpc
    ic = cap // 16
    n_groups = n_tiles * n_chunks

    idx_arr = np.zeros((P, n_groups * ic), np.int16)
    b_arr = np.full((P, n_tiles * nblk), float(P), ml_dtypes.bfloat16)
    cnt_arr = np.empty((P, n_groups), np.int32)

    bb, kk = np.nonzero(mask)
    gidx = midx[bb, kk]
    tile = bb // P
    lane = bb % P
    chunk = gidx // chunk_rows
    local = (gidx % chunk_rows).astype(np.int16)
    order = np.lexsort((chunk, tile))
    tile, lane, chunk, local = (tile[order], lane[order], chunk[order],
                                local[order])
    # group boundaries for (tile, chunk)
    gkey = tile * n_chunks + chunk
    starts = np.searchsorted(gkey, np.arange(n_groups))
    ends = np.searchsorted(gkey, np.arange(n_groups) + 1)

    for g in range(n_groups):
        s, e = starts[g], ends[g]
        n = e - s
        assert n <= cap, f"capacity overflow: group {g} has {n} > {cap}"
        t = g // n_chunks
        if t < DEPTH or not SKIP_PADS:
            # warm-up tiles: full-capacity row-0 padding initializes G bufs
            flat_idx = np.zeros(cap, np.int16)
            n_eff = cap
        elif n == 0:
            flat_idx = np.full(cap, -1, np.int16)
            flat_idx[0] = 0          # dead slot (lane P) keeps count >= 1
            n_eff = 1
        else:
            flat_idx = np.full(cap, -1, np.int16)
            n_eff = n
        flat_idx[:n] = local[s:e]
        cnt_arr[:, g] = n_eff
        flat_b = np.full(cap, float(P), np.float32)
        flat_b[:n] = lane[s:e]
        # wrapped int16 layout: flat j -> [j%16, j//16], replicated x8
        w16 = flat_idx.reshape(ic, 16).T
        idx_arr[:, g * ic:(g + 1) * ic] = np.tile(w16, (8, 1))
        # selection lane values: flat j -> block j//128, partition j%128
        c = g % n_chunks
        cols = flat_b.reshape(bpc, P).T          # [P, bpc]
        b_arr[:, t * nblk + c * bpc:(t * nblk + (c + 1) * bpc)] = (
            cols.astype(ml_dtypes.bfloat16))

    winv_arr = np.ascontiguousarray(
        winv.reshape(n_tiles, P).T.astype(np.float32))
    return idx_arr, b_arr, cnt_arr, winv_arr


def prep_inputs(features, neigh_idx, neigh_mask):
    import ml_dtypes

    features_bf = np.ascontiguousarray(
        np.asarray(features, dtype=np.float32).astype(ml_dtypes.bfloat16))
    neigh_idx = np.asarray(neigh_idx).astype(np.int64)
    neigh_mask = np.asarray(neigh_mask).astype(bool)

    winv = (1.0 / np.maximum(neigh_mask.sum(-1), 1)).astype(np.float32)

    pad = B_PAD - BATCH
    midx = np.concatenate(
        [neigh_idx, np.zeros((pad, K), np.int64)], axis=0).astype(np.int32)
    mask = np.concatenate([neigh_mask, np.zeros((pad, K), bool)], axis=0)
    winv = np.concatenate([winv, np.ones(pad, np.float32)])

    iota = np.tile(np.arange(P, dtype=np.float32),
                   (P, 1)).astype(ml_dtypes.bfloat16)

    in_maps = []
    for c in range(N_CORES):
        sl = slice(c * B_LOC, (c + 1) * B_LOC)
        idx_arr, b_arr, cnt_arr, winv_arr = pack_core(
            midx[sl], mask[sl], winv[sl],
            TILES_PER_CORE, CAP, CHUNK_ROWS, N_CHUNKS)
        in_maps.append({
            "features": features_bf,
            "idx": idx_arr,
            "bidx": b_arr,
            "iota": iota,
            "winv": winv_arr,
            "cnt": cnt_arr,
        })
    return in_maps


def kernel(features, neigh_idx, neigh_mask):
    from concourse.bass_utils import run_bass_kernel_spmd

    nc = get_program()
    in_maps = prep_inputs(features, neigh_idx, neigh_mask)
    res = run_bass_kernel_spmd(nc, in_maps, list(range(N_CORES)))
    full = np.concatenate(
        [res.results[c]["out"] for c in range(N_CORES)], axis=0)
    return full[:BATCH]


# revision 18
# speedup vs baseline: 2.2118x; 2.2118x over previous
"""Masked mean neighbor aggregation (GNN message passing) on 8 TRN2 cores.

Strategy (per spec sharding hint): batch dim sharded across 8 cores, feature
table replicated in each core's DRAM (bf16, host-converted).

The kernel is descriptor-rate bound: dma_gather costs ~21 ns per descriptor
per DMA engine (16 engines, ~1.3 ns aggregate) regardless of row bytes, and
descriptor count per gather instruction equals its STATIC capacity (the
deployed ucode fetches pad slots like real ones).  So the optimization
currency is the total static gather capacity.

Device algorithm, per core, per 128-row output tile:
  - The table is split into 4 row-range chunks (25000 rows each) so row
    indices fit dma_gather's int16 index format.  The host compacts each
    tile's unmasked (row, neighbor) slots by chunk into fixed-capacity flat
    index lists (pads point at spread-out rows to avoid HBM hotspots).
  - To shrink the static capacities, the host REPACKS batch rows into tiles
    (a 4-d bin-packing over per-chunk slot counts): N_HEAVY tiles with
    512-slot-per-chunk capacity absorb the high-degree rows, the remaining
    tiles only need 384.  This cuts descriptors/tile from 2048 to ~1664.
    The host un-permutes the output rows afterwards.  If packing ever fails
    (it should not for this input distribution), it falls back to uniform
    512 capacity.
  - 4 dma_gather instructions (one per chunk, on 4 parallel SWDGE queues)
    pull the 256 B bf16 feature rows into an SBUF tile G.
  - The slots land in a data-dependent order, so the host also ships a tiny
    per-slot target-lane array; the vector engine expands it on device into
    one-hot bf16 selection matrices (pad slots get lane 128 -> all-zero
    column).
  - PE computes psum[b, d] = sum_slots sel[slot, b] * G[slot, d] with PSUM
    accumulation over the tile's slot-blocks: simultaneously the masked sum
    and the reordering (bf16 inputs, fp32 accumulate).
  - The scalar engine scales by 1/max(count,1) (host-precomputed) and the
    result is stored as fp32.

Everything is raw bacc with manual semaphores (the Tile layer does not know
dma_gather's DMA semantics).
"""

from contextlib import ExitStack

import numpy as np

N_NODES = 100000
D_FEAT = 128
BATCH = 50000
K = 25
N_CORES = 8
P = 128

N_CHUNKS = 4
CHUNK_ROWS = 25000           # N_NODES / N_CHUNKS, < 32768 so int16-safe
TILES_PER_CORE = 49          # ceil(50000 / 8 / 128)
B_LOC = TILES_PER_CORE * P   # 6272
B_PAD = B_LOC * N_CORES      # 50176
DEPTH = 8                    # software pipeline depth (G/SEL/PSUM buffers)

N_HEAVY = 10
CAP_HEAVY = 512
CAP_LIGHT = 384
# per-tau per-chunk static gather capacity (same for all 4 chunks)
CAP_PROFILE = (CAP_HEAVY,) * N_HEAVY + (CAP_LIGHT,) * (TILES_PER_CORE - N_HEAVY)
MAX_BPC = CAP_HEAVY // P

_prog_cache = {}


def _build_program(n_rows, chunk_rows, n_chunks, d, n_tiles, cap_profile,
                   reps=1):
    import concourse.bass as bass
    import concourse.bacc as bacc
    import concourse.mybir as mybir
    from concourse.library_config import mlp

    bpc = [c // P for c in cap_profile]          # G blocks per chunk, per tau
    nblk = [n_chunks * b for b in bpc]           # selection blocks per tau
    ic = [c // 16 for c in cap_profile]          # idx cols per gather, per tau
    idx_off = np.concatenate([[0], np.cumsum([n_chunks * i for i in ic])])
    b_off = np.concatenate([[0], np.cumsum(nblk)])
    idx_cols = int(idx_off[-1])
    b_cols = int(b_off[-1])

    nc = bacc.Bacc("TRN2", target_bir_lowering=False, debug=False,
                   num_devices=N_CORES, num_swdge_queues=n_chunks)

    ftab = nc.dram_tensor("features", [n_rows, d], mybir.dt.bfloat16,
                          kind="ExternalInput")
    idx_d = nc.dram_tensor("idx", [P, idx_cols], mybir.dt.int16,
                           kind="ExternalInput")
    b_d = nc.dram_tensor("bidx", [P, b_cols], mybir.dt.bfloat16,
                         kind="ExternalInput")
    iota_d = nc.dram_tensor("iota", [P, P], mybir.dt.bfloat16,
                            kind="ExternalInput")
    winv_d = nc.dram_tensor("winv", [P, n_tiles], mybir.dt.float32,
                            kind="ExternalInput")
    out_d = nc.dram_tensor("out", [n_tiles * P, d], mybir.dt.float32,
                           kind="ExternalOutput")

    with ExitStack() as stack:
        block = stack.enter_context(nc.Block())
        ec = stack.enter_context
        idx_sb = ec(nc.sbuf_tensor("idx_sb", [P, idx_cols], mybir.dt.int16))
        b_sb = ec(nc.sbuf_tensor("b_sb", [P, b_cols], mybir.dt.bfloat16))
        iota_sb = ec(nc.sbuf_tensor("iota_sb", [P, P], mybir.dt.bfloat16))
        winv_sb = ec(nc.sbuf_tensor("winv_sb", [P, n_tiles],
                                    mybir.dt.float32))
        G = [ec(nc.sbuf_tensor(f"g{r}", [P, n_chunks * MAX_BPC, d],
                               mybir.dt.bfloat16)) for r in range(DEPTH)]
        SEL = [ec(nc.sbuf_tensor(f"sel{r}", [P, n_chunks * MAX_BPC, P],
                                 mybir.dt.bfloat16)) for r in range(DEPTH)]
        OSB = [ec(nc.sbuf_tensor(f"osb{r}", [P, d], mybir.dt.float32))
               for r in range(DEPTH)]
        PS = [ec(nc.psum_tensor(f"ps{r}", [P, d], mybir.dt.float32))
              for r in range(DEPTH)]
        io = ec(nc.semaphore("io"))
        gq = [[ec(nc.semaphore(f"gq{c}_{r}")) for r in range(DEPTH)]
              for c in range(n_chunks)]
        selg = ec(nc.semaphore("selg"))
        mmd = ec(nc.semaphore("mmd"))
        scd = ec(nc.semaphore("scd"))
        sto = [ec(nc.semaphore(f"sto{r}")) for r in range(DEPTH)]

        @block.sync
        def _(sync: bass.BassEngine):
            sync.dma_start(idx_sb[:], idx_d[:]).then_inc(io, 16)
            sync.dma_start(b_sb[:], b_d[:]).then_inc(io, 16)
            sync.dma_start(iota_sb[:], iota_d[:]).then_inc(io, 16)
            sync.dma_start(winv_sb[:], winv_d[:]).then_inc(io, 16)
            for tau in range(reps * n_tiles):
                t = tau % n_tiles
                sync.wait_ge(scd, tau + 1)
                sync.dma_start(out_d[t * P:(t + 1) * P, :],
                               OSB[tau % DEPTH][:]).then_inc(
                                   sto[tau % DEPTH], 16)

        @block.gpsimd
        def _(gpsimd: bass.BassGpSimd):
            gpsimd.load_library(mlp)
            gpsimd.wait_ge(io, 64)
            for tau in range(reps * n_tiles):
                t = tau % n_tiles
                if tau >= DEPTH:
                    gpsimd.wait_ge(mmd, tau - DEPTH + 1)  # G[tau%DEPTH] free
                gt = G[tau % DEPTH]
                cap, bp, icc = cap_profile[t], bpc[t], ic[t]
                for c in range(n_chunks):
                    src = ftab[c * chunk_rows:(c + 1) * chunk_rows, :]
                    lo = int(idx_off[t]) + c * icc
                    idxs = idx_sb[:, lo:lo + icc]
                    gpsimd.dma_gather(
                        gt[:, c * bp:(c + 1) * bp, :], src, idxs,
                        cap, cap, d, queue_num=c,
                    ).then_inc(gq[c][tau % DEPTH], 16)

        @block.vector
        def _(vector: bass.BassVectorEngine):
            vector.wait_ge(io, 64)
            iv = iota_sb.ap()
            for tau in range(reps * n_tiles):
                t = tau % n_tiles
                if tau >= DEPTH:
                    vector.wait_ge(mmd, tau - DEPTH + 1)  # sel free
                nb = nblk[t]
                iota_bc = bass.AP(iv.tensor, iv.offset,
                                  [iv.ap[0], [0, nb], iv.ap[1]])
                st = SEL[tau % DEPTH]
                bv = b_sb[:, int(b_off[t]):int(b_off[t]) + nb]
                b_bc = bass.AP(bv.tensor, bv.offset,
                               [bv.ap[0], bv.ap[1], [0, P]])
                vector.tensor_tensor(
                    out=st[:, 0:nb, :], in0=iota_bc, in1=b_bc,
                    op=mybir.AluOpType.is_equal,
                ).then_inc(selg, 1)

        @block.scalar
        def _(scalar: bass.BassEngine):
            scalar.wait_ge(io, 64)
            for tau in range(reps * n_tiles):
                t = tau % n_tiles
                scalar.wait_ge(mmd, tau + 1)     # psum[tau%DEPTH] ready
                if tau >= DEPTH:
                    scalar.wait_ge(sto[tau % DEPTH],
                                   16 * (tau // DEPTH))  # OSB free
                scalar.mul(OSB[tau % DEPTH][:], PS[tau % DEPTH][:],
                           winv_sb[:, t:t + 1]).then_inc(scd, 1)

        @block.tensor
        def _(tensor: bass.BassEngine):
            for tau in range(reps * n_tiles):
                t = tau % n_tiles
                for c in range(n_chunks):
                    tensor.wait_ge(gq[c][tau % DEPTH],
                                   16 * (tau // DEPTH + 1))
                tensor.wait_ge(selg, tau + 1)
                if tau >= DEPTH:
                    tensor.wait_ge(scd, tau - DEPTH + 1)  # psum drained
                gt, st, pst = (G[tau % DEPTH], SEL[tau % DEPTH],
                               PS[tau % DEPTH])
                nb = nblk[t]
                for blk in range(nb):
                    inst = nc.tensor.matmul(
                        pst[:], st[:, blk, :], gt[:, blk, :],
                        start=(blk == 0), stop=(blk == nb - 1),
                    )
                inst.then_inc(mmd, 1)

    nc.compile()
    return nc


def get_program(reps=1, profile=CAP_PROFILE):
    key = (tuple(profile), reps)
    if key not in _prog_cache:
        _prog_cache[key] = _build_program(
            N_NODES, CHUNK_ROWS, N_CHUNKS, D_FEAT, TILES_PER_CORE,
            tuple(profile), reps)
    return _prog_cache[key]


def _pack_tiles(cc, profile, max_swaps=20000):
    """Assign b_loc rows to tiles so per-(tile,chunk) counts fit `profile`.

    Band-mix heavy pool + balanced deal + chunk-repair swaps.
    cc: [b_loc, 4] int — per-row unmasked slot count per chunk.
    Returns rowmap [b_loc] (rowmap[packed_position] = original local row)
    or None if packing failed.
    """
    b_loc = cc.shape[0]
    n_tiles = len(profile)
    caps = np.array(profile, np.int64)[:, None] * np.ones((1, N_CHUNKS),
                                                          np.int64)
    nh = int(np.sum(np.array(profile) == CAP_HEAVY))
    if nh == n_tiles:
        return np.arange(b_loc, dtype=np.int64)
    k = cc.sum(1)
    total = k.sum()
    order = np.argsort(-k, kind="stable")
    th_target = int(total - (n_tiles - nh) * (CAP_LIGHT * N_CHUNKS - 48))
    # heavy pool: top rows + rows near the needed average, exactly nh*128
    n_heavy_rows = nh * P
    n_top = n_heavy_rows // 2
    top = order[:n_top]
    rest_sorted = order[n_top:]
    need = th_target - int(k[top].sum())
    n_rem = n_heavy_rows - n_top
    avg_need = need / max(n_rem, 1)
    kr = k[rest_sorted]
    sel = np.argsort(np.abs(kr - avg_need), kind="stable")[:n_rem]
    heavy_rows = np.concatenate([top, rest_sorted[sel]])
    mask_h = np.zeros(b_loc, bool)
    mask_h[heavy_rows] = True
    light_rows = np.nonzero(~mask_h)[0]

    assign = np.full(b_loc, -1, np.int64)
    loads = np.zeros((n_tiles, N_CHUNKS), np.int64)
    fill = np.zeros(n_tiles, np.int64)

    def deal(rows, tiles):
        rows = rows[np.argsort(-k[rows], kind="stable")]
        for r in rows:
            cand = tiles[fill[tiles] < P]
            frac = ((loads[cand] + cc[r]) / caps[cand]).max(1)
            t = cand[np.argmin(frac)]
            assign[r] = t
            loads[t] += cc[r]
            fill[t] += 1

    deal(heavy_rows, np.arange(nh))
    deal(light_rows, np.arange(nh, n_tiles))

    rows_of = [np.nonzero(assign == t)[0] for t in range(n_tiles)]
    for _ in range(max_swaps):
        over = loads - caps
        t, c = np.unravel_index(np.argmax(over), over.shape)
        if over[t, c] <= 0:
            break
        rt = rows_of[t]
        r1 = rt[np.argmax(cc[rt, c])]
        best = None
        d1 = cc[r1]
        for t2 in range(n_tiles):
            if t2 == t:
                continue
            head2 = caps[t2] - loads[t2]
            rt2 = rows_of[t2]
            delta = d1 - cc[rt2]
            ok = (delta <= head2).all(1)
            if not ok.any():
                continue
            newt = loads[t] - d1 + cc[rt2]
            scoret = (newt - caps[t]).max(1)
            scoret[~ok] = 1 << 30
            j = int(np.argmin(scoret))
            if best is None or scoret[j] < best[0]:
                best = (scoret[j], t2, rt2[j])
        if best is None:
            return None
        _, t2, r2 = best
        loads[t] += cc[r2] - cc[r1]
        loads[t2] += cc[r1] - cc[r2]
        assign[r1], assign[r2] = t2, t
        rows_of[t] = np.nonzero(assign == t)[0]
        rows_of[t2] = np.nonzero(assign == t2)[0]
    if ((loads - caps) > 0).any() or (fill != P).any():
        return None
    return np.argsort(assign, kind="stable")


def pack_core(midx, mask, winv, profile):
    """Compact one core's slots into gather/selection arrays.

    midx: [b_loc, K] int32 global row idx; mask: [b_loc, K] bool;
    winv: [b_loc] f32.  Returns (idx_arr, b_arr, winv_arr, rowmap) or None.
    """
    import ml_dtypes

    n_tiles = TILES_PER_CORE
    b_loc = midx.shape[0]
    chunk_of = midx // CHUNK_ROWS
    chunk_cnt = np.zeros((b_loc, N_CHUNKS), np.int64)
    for c in range(N_CHUNKS):
        chunk_cnt[:, c] = ((chunk_of == c) & mask).sum(1)

    rowmap = _pack_tiles(chunk_cnt, profile)
    if rowmap is None:
        return None

    bpc = [c // P for c in profile]
    nblk = [N_CHUNKS * b for b in bpc]
    ic = [c // 16 for c in profile]
    idx_off = np.concatenate([[0], np.cumsum([N_CHUNKS * i for i in ic])])
    b_off = np.concatenate([[0], np.cumsum(nblk)])

    idx_arr = np.zeros((P, int(idx_off[-1])), np.int16)
    b_arr = np.full((P, int(b_off[-1])), float(P), ml_dtypes.bfloat16)

    pmidx = midx[rowmap]
    pmask = mask[rowmap]
    bb, kk = np.nonzero(pmask)
    gidx = pmidx[bb, kk]
    tile = bb // P
    lane = bb % P
    chunk = gidx // CHUNK_ROWS
    local = (gidx % CHUNK_ROWS).astype(np.int16)
    order = np.lexsort((chunk, tile))
    tile, lane, chunk, local = (tile[order], lane[order], chunk[order],
                                local[order])
    gkey = tile * N_CHUNKS + chunk
    starts = np.searchsorted(gkey, np.arange(n_tiles * N_CHUNKS))
    ends = np.searchsorted(gkey, np.arange(n_tiles * N_CHUNKS) + 1)

    # pads spread across rows (avoids an HBM same-row hotspot)
    pad_rows = (np.arange(CAP_HEAVY, dtype=np.int64) * 61) % CHUNK_ROWS

    for g in range(n_tiles * N_CHUNKS):
        s, e = starts[g], ends[g]
        n = e - s
        t, c = g // N_CHUNKS, g % N_CHUNKS
        cap = profile[t]
        if n > cap:
            return None
        icc = ic[t]
        flat_idx = pad_rows[:cap].astype(np.int16)
        flat_idx[:n] = local[s:e]
        flat_b = np.full(cap, float(P), np.float32)
        flat_b[:n] = lane[s:e]
        # wrapped int16 layout: flat j -> [j%16, j//16], replicated x8
        w16 = flat_idx.reshape(icc, 16).T
        lo = int(idx_off[t]) + c * icc
        idx_arr[:, lo:lo + icc] = np.tile(w16, (8, 1))
        # selection lane values: flat j -> block j//128, partition j%128
        cols = flat_b.reshape(bpc[t], P).T          # [P, bpc]
        bo = int(b_off[t]) + c * bpc[t]
        b_arr[:, bo:bo + bpc[t]] = cols.astype(ml_dtypes.bfloat16)

    pwinv = winv[rowmap]
    winv_arr = np.ascontiguousarray(
        pwinv.reshape(n_tiles, P).T.astype(np.float32))
    return idx_arr, b_arr, winv_arr, rowmap


def prep_inputs(features, neigh_idx, neigh_mask, profile=CAP_PROFILE):
    """Returns (in_maps, rowmaps) or None if the profile cannot fit."""
    import ml_dtypes

    features_bf = np.ascontiguousarray(
        np.asarray(features, dtype=np.float32).astype(ml_dtypes.bfloat16))
    neigh_idx = np.asarray(neigh_idx).astype(np.int64)
    neigh_mask = np.asarray(neigh_mask).astype(bool)

    winv = (1.0 / np.maximum(neigh_mask.sum(-1), 1)).astype(np.float32)

    pad = B_PAD - BATCH
    midx = np.concatenate(
        [neigh_idx, np.zeros((pad, K), np.int64)], axis=0).astype(np.int32)
    mask = np.concatenate([neigh_mask, np.zeros((pad, K), bool)], axis=0)
    winv = np.concatenate([winv, np.ones(pad, np.float32)])

    iota = np.tile(np.arange(P, dtype=np.float32),
                   (P, 1)).astype(ml_dtypes.bfloat16)

    in_maps, rowmaps = [], []
    for c in range(N_CORES):
        sl = slice(c * B_LOC, (c + 1) * B_LOC)
        packed = pack_core(midx[sl], mask[sl], winv[sl], profile)
        if packed is None:
            return None
        idx_arr, b_arr, winv_arr, rowmap = packed
        in_maps.append({
            "features": features_bf,
            "idx": idx_arr,
            "bidx": b_arr,
            "iota": iota,
            "winv": winv_arr,
        })
        rowmaps.append(rowmap)
    return in_maps, rowmaps


def _profile_ladder():
    for nh in (N_HEAVY, 14, TILES_PER_CORE):
        yield (CAP_HEAVY,) * nh + (CAP_LIGHT,) * (TILES_PER_CORE - nh)


def kernel(features, neigh_idx, neigh_mask):
    from concourse.bass_utils import run_bass_kernel_spmd

    prepped = profile = None
    for profile in _profile_ladder():
        prepped = prep_inputs(features, neigh_idx, neigh_mask, profile)
        if prepped is not None:
            break
    in_maps, rowmaps = prepped
    nc = get_program(profile=profile)
    res = run_bass_kernel_spmd(nc, in_maps, list(range(N_CORES)))
    full = np.empty((B_PAD, D_FEAT), np.float32)
    for c in range(N_CORES):
        full[c * B_LOC + rowmaps[c]] = res.results[c]["out"]
    return full[:BATCH]
